# revision 31
# baseline (speedup 1.0000x reference)
"""Trainium2 Bass kernel for BiomechanicGATHead (all-bf16 pipeline).

Math restructure (exact, done host-side in float64):
    h  = gelu(x @ W1 + b1)                       [R,256]
    GAT(n, adj, Wg, bg) = gelu((softmax(adj) @ n_nodes) @ Wg + bg) + n
  Flattened over (node, feat) the GAT linear is M = kron(softmax(adj).T, Wg).
  GAT1 is folded into the preceding linear (W2K1 = W2 @ M1), with b2
  deferred into downstream biases so residual adds consume raw PSUM:
    t1  = gelu(h @ W2K1 + bK1)
    m1  = t1 + h @ W2                ("n1 - b2")
    z2  = m1 @ M2 + bG2 ;  t2 = gelu(z2) ;  m2 = t2 + m1
    out = m2 @ C + bC                with C = kron(I17, Wc)

Precision: every matmul runs in bf16 (measured end-to-end rel-L2 ~4.5e-3
vs the f64 oracle; the harness gate is 2e-2).  bf16 streams at the same
1 cycle/row as f32r on the PE, but its 2-byte weight loads (~110 ns)
hide fully behind the 512-row stream (~213 ns), dropping the measured
matmul cadence from 275 ns to ~220 ns.  fp8 DoubleRow was tried and
measured at 1 cycle/row on this hardware (no gain), so it is not used.

544 is padded to 640 = 5*128 with zero rows/cols (pads never affect the
output because all padded weight ROWS are zero).

Engine schedule per 512-row tile (software-pipelined): iteration t puts
L1(t), GAT2(t-1), L5(t-2), then L2b/L2a(t) on the PE.  GAT2/L5 consume
products made a full iteration earlier, and running them before L2(t)
means every PSUM group-start reuses a bank freed an iteration ago
(group-starts whose bank was freed by an add ~27ns earlier cost the PE
a ~432ns pipeline restart).
  Scalar: 12 gelus (+1 dummy at boot to pull the 1.28us gelu
  ACT_TABLE_LOAD off tile 0's critical path);  Vector: 10 residual
  adds + 1 bias add;  t1 stays f32 so the m1 add reads uniform-f32
  inputs (mixed bf16/f32 tensor_tensor measured a ~1.9us slow path).
PSUM: shared 7-deep pp rotation + po(1) = 8 banks.  C is padded to 128
output columns so the PE never switches tile config (128x34 <-> 128x128
switches measured ~538ns each).
DMA: x loads + the three big weight slabs (issued right after xt0) on
sync's HWDGE queue -- it carries no dependency-waiting instructions, so
nothing blocks head-of-line; small constants on gpsimd's SWDGE queue
(~1us/descriptor); output stores on gpsimd so a store waiting on its
bias add can never delay an x load.  Issuing weight DMAs from scalar's
queue was measured to delay tile 0's gelus ~4us.

Sharding: pure data parallel, 65536 rows split as 8192 rows x 8 cores.
"""

import numpy as np
import ml_dtypes

import concourse.bass as bass
import concourse.mybir as mybir
import concourse.tile as tile
from concourse import bacc
from concourse.bass_utils import run_bass_kernel_spmd

N_CORES = 8
D, HID, NN, ND = 128, 256, 17, 32
F = NN * ND          # 544
KC = 5               # 128-chunks covering the padded feature dim
FP = KC * 128        # 640
OUTW = NN * 2        # 34
B, W = 16, 4096
ROWS = B * W         # 65536
R_CORE = ROWS // N_CORES   # 8192
TILE_N = 512
N_TILES = R_CORE // TILE_N  # 16

f32 = mybir.dt.float32
bf16 = mybir.dt.bfloat16
GELU = mybir.ActivationFunctionType.Gelu

np_bf16 = ml_dtypes.bfloat16


def _prep_constants(W1, b1, W2, b2, adj1, Wg1, bg1, adj2, Wg2, bg2, Wc, bc):
    """Fold the network into the fused layers; return device-layout arrays."""
    d = {}
    f64 = np.float64

    def softmax(a):
        a = a.astype(f64)
        e = np.exp(a - a.max(axis=-1, keepdims=True))
        return e / e.sum(axis=-1, keepdims=True)

    A1 = softmax(adj1)
    A2 = softmax(adj2)
    M1 = np.kron(A1.T, Wg1.astype(f64))          # [544, 544]
    M2 = np.kron(A2.T, Wg2.astype(f64))          # [544, 544]
    C = np.kron(np.eye(NN), Wc.astype(f64))      # [544, 34]

    W2K1 = W2.astype(f64) @ M1                   # [256, 544]
    bK1 = b2.astype(f64) @ M1 + np.tile(bg1.astype(f64), NN)   # [544]
    bG2 = b2.astype(f64) @ M2 + np.tile(bg2.astype(f64), NN)   # [544]
    bC = b2.astype(f64) @ C + np.tile(bc.astype(f64), NN)      # [34]

    def padcols(a, w):
        out = np.zeros((a.shape[0], w), f64)
        out[:, : a.shape[1]] = a
        return out

    def padrows(a, h):
        out = np.zeros((h,) + a.shape[1:], f64)
        out[: a.shape[0]] = a
        return out

    W2p = padcols(W2.astype(f64), FP)            # [256, 640]
    W2K1p = padcols(W2K1, FP)                    # [256, 640]
    M2p = padrows(padcols(M2, FP), FP)           # [640, 640]
    Cp = padrows(C, FP)                          # [640, 34]
    bK1p = padrows(bK1, FP)                      # [640]
    bG2p = padrows(bG2, FP)                      # [640]

    def asb(a):  # -> bf16 device array
        return np.ascontiguousarray(np.asarray(a, dtype=np.float32).astype(np_bf16))

    asf = lambda a: np.ascontiguousarray(a, dtype=np.float32)

    # SBUF layouts: partition dim first; K-chunks as middle axis.
    d["w1"] = asb(W1)                                            # [128, 256]
    d["w2"] = asb(W2p.reshape(2, 128, FP).transpose(1, 0, 2))    # [128, 2, 640]
    d["w2k1"] = asb(W2K1p.reshape(2, 128, FP).transpose(1, 0, 2))
    d["m2"] = asb(M2p.reshape(KC, 128, FP).transpose(1, 0, 2))   # [128, 5, 640]
    # C is padded to 128 output columns: a 34-wide lhsT makes the PE use a
    # 128x34 tile config, and the config switch back to 128x128 for the
    # next stage was measured to cost ~538ns every iteration.
    Cp128 = np.zeros((FP, 128), f64)
    Cp128[:, :OUTW] = Cp
    d["cw"] = asb(Cp128.reshape(KC, 128, 128).transpose(1, 0, 2))  # [128, 5, 128]
    d["b1"] = asf(b1.astype(f64).reshape(2, 128).T)              # [128, 2]
    d["bk1"] = asf(bK1p.reshape(KC, 128).T)                      # [128, 5]
    d["bg2"] = asf(bG2p.reshape(KC, 128).T)                      # [128, 5]
    d["bc"] = asf(bC.reshape(OUTW, 1))                           # [34, 1]
    return d


def _build_nc():
    """Build the per-core Bass program (same NEFF on all 8 cores)."""
    nc = bacc.Bacc("TRN2", target_bir_lowering=False, debug=False)

    xT = nc.dram_tensor("xT", [D, R_CORE], bf16, kind="ExternalInput").ap()
    w1 = nc.dram_tensor("w1", [128, HID], bf16, kind="ExternalInput").ap()
    w2 = nc.dram_tensor("w2", [128, 2, FP], bf16, kind="ExternalInput").ap()
    w2k1 = nc.dram_tensor("w2k1", [128, 2, FP], bf16, kind="ExternalInput").ap()
    m2 = nc.dram_tensor("m2", [128, KC, FP], bf16, kind="ExternalInput").ap()
    cw = nc.dram_tensor("cw", [128, KC, 128], bf16, kind="ExternalInput").ap()
    b1 = nc.dram_tensor("b1", [128, 2], f32, kind="ExternalInput").ap()
    bk1 = nc.dram_tensor("bk1", [128, KC], f32, kind="ExternalInput").ap()
    bg2 = nc.dram_tensor("bg2", [128, KC], f32, kind="ExternalInput").ap()
    bc = nc.dram_tensor("bc", [OUTW, 1], f32, kind="ExternalInput").ap()
    outT = nc.dram_tensor("outT", [OUTW, R_CORE], f32, kind="ExternalOutput").ap()

    with tile.TileContext(nc) as tc:
        with (
            tc.tile_pool(name="consts", bufs=1) as consts,
            tc.tile_pool(name="acts", bufs=2) as acts,
            tc.tile_pool(name="xio", bufs=3) as xio,
            tc.tile_pool(name="ps", bufs=1, space=bass.MemorySpace.PSUM) as ps,
        ):
            # All weights stream on GPSIMD's DMA queue (gpsimd is otherwise
            # idle): putting them on scalar's queue was measured to delay
            # tile 0's gelus ~4us (each DMA issue costs ~700ns of
            # engine-sequencer time), and sync's queue must stay free so
            # xt(0) issues immediately.
            # gpsimd queue: the weights tile 0 needs first (L1/L2).
            # Small constants ride gpsimd's SWDGE queue (~1us/descriptor but
            # tiny transfers); the three big slabs go on scalar's HWDGE
            # queue, whose 3 issue slots (~2.4us) clear before tile 0's
            # first gelu, so the scalar engine is not delayed and m2 lands
            # ~6us earlier than on the SWDGE queue.
            w1s = consts.tile([128, HID], bf16)
            nc.gpsimd.dma_start(w1s, w1)
            b1s = consts.tile([128, 2], f32)
            nc.gpsimd.dma_start(b1s, b1)
            bk1s = consts.tile([128, KC], f32)
            nc.gpsimd.dma_start(bk1s, bk1)
            bg2s = consts.tile([128, KC], f32)
            nc.gpsimd.dma_start(bg2s, bg2)
            cws = consts.tile([128, KC, 128], bf16)
            nc.gpsimd.dma_start(cws, cw)
            bcs = consts.tile([OUTW, 1], f32)
            nc.gpsimd.dma_start(bcs, bc)

            # Dummy 1-element gelu at the head of the scalar queue: forces
            # the 1.28us ACT_TABLE_LOAD during the DMA dead-time instead of
            # on tile 0's critical path (measured at 13.2us otherwise).
            scr = consts.tile([1, 2], f32)
            nc.vector.memset(scr, 0)
            nc.scalar.activation(scr[0:1, 1:2], scr[0:1, 0:1], GELU)

            # Big slabs are issued on the sync queue right after xt(0)
            # (see the t==0 branch in the loop): sync carries no
            # dependency-waiting instructions, so they stream immediately
            # and in parallel with gpsimd's small constants, while the
            # scalar queue stays free to run tile 0's gelus on time.
            w2k1s = consts.tile([128, 2, FP], bf16)
            w2s = consts.tile([128, 2, FP], bf16)
            m2s_w = consts.tile([128, KC, FP], bf16)

            def emit_gat2(p):
                """GAT2 for tile p: z2 = m1@M2 (psum), t2 = gelu(z2 + bG2),
                m2 = t2 + m1.  Runs one iteration after its m1 was made, so
                m1 is long ready and the PE streams without stalls."""
                m1s, p_t = p
                t2s = acts.tile([128, KC, TILE_N], bf16, tag="t2s")
                m2s = acts.tile([128, KC, TILE_N], bf16, tag="m2s", bufs=3)
                for m in range(KC):
                    pz = ps.tile([128, TILE_N], f32, tag="pp", bufs=7,
                                 name=f"pz_{p_t}_{m}")
                    for k in range(KC):
                        nc.tensor.matmul(pz, m2s_w[:, k, bass.ts(m, 128)],
                                         m1s[:, k, :],
                                         start=(k == 0), stop=(k == KC - 1))
                    nc.scalar.activation(t2s[:, m, :], pz, GELU,
                                         bias=bg2s[:, m : m + 1])
                    nc.vector.tensor_add(m2s[:, m, :], t2s[:, m, :], m1s[:, m, :])
                return m2s

            def emit_l5(p, store_on_sync=False):
                """out = m2 @ C + bC for tile p (two iterations late)."""
                m2s, p_sl, p_t = p
                po = ps.tile([128, TILE_N], f32, tag="po", bufs=1, name=f"po_{p_t}")
                for k in range(KC):
                    nc.tensor.matmul(po, cws[:, k, :], m2s[:, k, :],
                                     start=(k == 0), stop=(k == KC - 1))
                ot = xio.tile([OUTW, TILE_N], f32, tag="ot", name=f"ot_{p_t}")
                nc.vector.tensor_scalar_add(ot, po[0:OUTW, :], bcs)
                q = nc.sync if store_on_sync else nc.gpsimd
                q.dma_start(outT[:, p_sl], ot)

            prev1 = None   # tile awaiting GAT2
            prev2 = None   # tile awaiting L5
            for t in range(N_TILES):
                sl = bass.ts(t, TILE_N)

                xt = xio.tile([D, TILE_N], bf16, tag="xt", name=f"xt_{t}")
                nc.sync.dma_start(xt, xT[:, sl])
                if t == 0:
                    nc.sync.dma_start(w2k1s, w2k1)
                    nc.sync.dma_start(w2s, w2)
                    # m2 is split per k-chunk so GAT2(0)'s first groups can
                    # start as soon as their chunk lands instead of waiting
                    # for the whole 800KB slab (~1.5us of warm-up stalls).
                    for kk in range(KC):
                        nc.sync.dma_start(m2s_w[:, kk, :], m2[:, kk, :])

                # L1: hT = gelu(W1.T @ xT + b1)   [2 chunks of 128]
                # L1's PSUM joins the shared 7-deep rotation: a dedicated
                # 2-bank tile left the rotation at depth 5, where GAT2's
                # group-start matmuls measurably wait (~430ns/iteration)
                # for the vector add freeing their bank.
                hs = acts.tile([128, 2, TILE_N], bf16, tag="hs")
                for c in range(2):
                    ph = ps.tile([128, TILE_N], f32, tag="pp", bufs=7,
                                 name=f"ph_{t}_{c}")
                    nc.tensor.matmul(ph, w1s[:, bass.ts(c, 128)], xt,
                                     start=True, stop=True)
                    nc.scalar.activation(hs[:, c, :], ph, GELU,
                                         bias=b1s[:, c : c + 1])

                # GAT2(t-1) and L5(t-2) run RIGHT AFTER L1(t), before L2(t):
                # this way every pz group-start's PSUM bank tenant (a
                # pn0/pt1 from the PREVIOUS iteration) was freed ~an
                # iteration ago, instead of by an m1 add ~27ns earlier --
                # each just-in-time wait cost the PE a ~432ns pipeline
                # restart per iteration.
                # Iteration 1 runs L2(1) BEFORE GAT2(0): GAT2(0) otherwise
                # stalls ~3us nibbling on the m2 slab as it streams in;
                # L2(1) needs only w2k1/w2 (already landed) and buys the
                # m2 transfer the whole L2 phase of cover.
                if prev1 is not None and t != 1:
                    m2s = emit_gat2(prev1)
                    if prev2 is not None:
                        emit_l5(prev2)
                    prev2 = (m2s, bass.ts(prev1[1], TILE_N), prev1[1])

                # L2b/L2a interleaved per output chunk:
                #   t1 = gelu(h @ W2K1 + bK1)   (GAT1 fused; t1 stays f32)
                #   m1 = t1 + h @ W2            (b2 deferred; bf16 out)
                t1s = acts.tile([128, KC, TILE_N], f32, tag="t1s")
                m1s = acts.tile([128, KC, TILE_N], bf16, tag="m1s", bufs=3)
                for m in range(KC):
                    pt1 = ps.tile([128, TILE_N], f32, tag="pp", bufs=7,
                                  name=f"pt1_{t}_{m}")
                    for k in range(2):
                        nc.tensor.matmul(pt1, w2k1s[:, k, bass.ts(m, 128)],
                                         hs[:, k, :], start=(k == 0), stop=(k == 1))
                    nc.scalar.activation(t1s[:, m, :], pt1, GELU,
                                         bias=bk1s[:, m : m + 1])
                    pn0 = ps.tile([128, TILE_N], f32, tag="pp", bufs=7,
                                  name=f"pn0_{t}_{m}")
                    for k in range(2):
                        nc.tensor.matmul(pn0, w2s[:, k, bass.ts(m, 128)],
                                         hs[:, k, :], start=(k == 0), stop=(k == 1))
                    nc.vector.tensor_add(m1s[:, m, :], t1s[:, m, :], pn0)

                if prev1 is not None and t == 1:
                    m2s = emit_gat2(prev1)
                    prev2 = (m2s, bass.ts(prev1[1], TILE_N), prev1[1])
                prev1 = (m1s, t)

            # flush the pipeline tail (last two stores ride sync's HWDGE
            # queue -- idle by now, and its completion latency is lower
            # than gpsimd's SWDGE, shortening the drain)
            m2s = emit_gat2(prev1)
            if prev2 is not None:
                emit_l5(prev2, store_on_sync=True)
            emit_l5((m2s, bass.ts(N_TILES - 1, TILE_N), N_TILES - 1),
                    store_on_sync=True)

    nc.compile()
    return nc


_NC_CACHE = None


def _run(inputs: dict, trace: bool = False):
    global _NC_CACHE
    if _NC_CACHE is None:
        _NC_CACHE = _build_nc()
    nc = _NC_CACHE

    x = np.ascontiguousarray(inputs["x"], dtype=np.float32)
    consts = _prep_constants(
        *(np.asarray(inputs[k], dtype=np.float32)
          for k in ("W1", "b1", "W2", "b2", "adj1", "Wg1", "bg1",
                    "adj2", "Wg2", "bg2", "Wc", "bc"))
    )

    xflat = x.reshape(ROWS, D)
    in_maps = []
    for i in range(N_CORES):
        shard = np.ascontiguousarray(
            xflat[i * R_CORE : (i + 1) * R_CORE].T.astype(np_bf16)
        )
        m = {"xT": shard}
        m.update(consts)
        in_maps.append(m)

    res = run_bass_kernel_spmd(nc, in_maps, core_ids=list(range(N_CORES)), trace=trace)
    parts = [np.asarray(r["outT"]).T for r in res.results]     # each [8192, 34]
    out = np.concatenate(parts, axis=0).reshape(B, W, NN, 2)
    return np.ascontiguousarray(out, dtype=np.float32), res


def kernel(**inputs) -> np.ndarray:
    out, _ = _run(inputs, trace=False)
    return out


# revision 32
# speedup vs baseline: 1.0152x; 1.0152x over previous
"""Trainium2 Bass kernel for BiomechanicGATHead (all-bf16 pipeline).

Math restructure (exact, done host-side in float64):
    h  = gelu(x @ W1 + b1)                       [R,256]
    GAT(n, adj, Wg, bg) = gelu((softmax(adj) @ n_nodes) @ Wg + bg) + n
  Flattened over (node, feat) the GAT linear is M = kron(softmax(adj).T, Wg).
  GAT1 is folded into the preceding linear (W2K1 = W2 @ M1), with b2
  deferred into downstream biases so residual adds consume raw PSUM:
    t1  = gelu(h @ W2K1 + bK1)
    m1  = t1 + h @ W2                ("n1 - b2")
    z2  = m1 @ M2 + bG2 ;  t2 = gelu(z2) ;  m2 = t2 + m1
    out = m2 @ C + bC                with C = kron(I17, Wc)

Precision: every matmul runs in bf16 (measured end-to-end rel-L2 ~4.5e-3
vs the f64 oracle; the harness gate is 2e-2).  bf16 streams at the same
1 cycle/row as f32r on the PE, but its 2-byte weight loads (~110 ns)
hide fully behind the 512-row stream (~213 ns), dropping the measured
matmul cadence from 275 ns to ~220 ns.  fp8 DoubleRow was tried and
measured at 1 cycle/row on this hardware (no gain), so it is not used.

544 is padded to 640 = 5*128 with zero rows/cols (pads never affect the
output because all padded weight ROWS are zero).

Engine schedule per 512-row tile (software-pipelined): iteration t puts
L1(t), GAT2(t-1), L5(t-2), then L2b/L2a(t) on the PE.  GAT2/L5 consume
products made a full iteration earlier, and running them before L2(t)
means every PSUM group-start reuses a bank freed an iteration ago
(group-starts whose bank was freed by an add ~27ns earlier cost the PE
a ~432ns pipeline restart).
  Scalar: 12 gelus (+1 dummy at boot to pull the 1.28us gelu
  ACT_TABLE_LOAD off tile 0's critical path);  Vector: 10 residual
  adds + 1 bias add;  t1 stays f32 so the m1 add reads uniform-f32
  inputs (mixed bf16/f32 tensor_tensor measured a ~1.9us slow path).
PSUM: shared 7-deep pp rotation + po(1) = 8 banks.  C is padded to 128
output columns so the PE never switches tile config (128x34 <-> 128x128
switches measured ~538ns each).
DMA: x loads + the three big weight slabs (issued right after xt0) on
sync's HWDGE queue -- it carries no dependency-waiting instructions, so
nothing blocks head-of-line; small constants on gpsimd's SWDGE queue
(~1us/descriptor); output stores on gpsimd so a store waiting on its
bias add can never delay an x load.  Issuing weight DMAs from scalar's
queue was measured to delay tile 0's gelus ~4us.

Sharding: pure data parallel, 65536 rows split as 8192 rows x 8 cores.
"""

import numpy as np
import ml_dtypes

import concourse.bass as bass
import concourse.mybir as mybir
import concourse.tile as tile
from concourse import bacc
from concourse.bass_utils import run_bass_kernel_spmd

N_CORES = 8
D, HID, NN, ND = 128, 256, 17, 32
F = NN * ND          # 544
KC = 5               # 128-chunks covering the padded feature dim
FP = KC * 128        # 640
OUTW = NN * 2        # 34
B, W = 16, 4096
ROWS = B * W         # 65536
R_CORE = ROWS // N_CORES   # 8192
TILE_N = 512
N_TILES = R_CORE // TILE_N  # 16

f32 = mybir.dt.float32
bf16 = mybir.dt.bfloat16
GELU = mybir.ActivationFunctionType.Gelu

np_bf16 = ml_dtypes.bfloat16


def _prep_constants(W1, b1, W2, b2, adj1, Wg1, bg1, adj2, Wg2, bg2, Wc, bc):
    """Fold the network into the fused layers; return device-layout arrays."""
    d = {}
    f64 = np.float64

    def softmax(a):
        a = a.astype(f64)
        e = np.exp(a - a.max(axis=-1, keepdims=True))
        return e / e.sum(axis=-1, keepdims=True)

    A1 = softmax(adj1)
    A2 = softmax(adj2)
    M1 = np.kron(A1.T, Wg1.astype(f64))          # [544, 544]
    M2 = np.kron(A2.T, Wg2.astype(f64))          # [544, 544]
    C = np.kron(np.eye(NN), Wc.astype(f64))      # [544, 34]

    W2K1 = W2.astype(f64) @ M1                   # [256, 544]
    bK1 = b2.astype(f64) @ M1 + np.tile(bg1.astype(f64), NN)   # [544]
    bG2 = b2.astype(f64) @ M2 + np.tile(bg2.astype(f64), NN)   # [544]
    bC = b2.astype(f64) @ C + np.tile(bc.astype(f64), NN)      # [34]

    def padcols(a, w):
        out = np.zeros((a.shape[0], w), f64)
        out[:, : a.shape[1]] = a
        return out

    def padrows(a, h):
        out = np.zeros((h,) + a.shape[1:], f64)
        out[: a.shape[0]] = a
        return out

    W2p = padcols(W2.astype(f64), FP)            # [256, 640]
    W2K1p = padcols(W2K1, FP)                    # [256, 640]
    M2p = padrows(padcols(M2, FP), FP)           # [640, 640]
    Cp = padrows(C, FP)                          # [640, 34]
    bK1p = padrows(bK1, FP)                      # [640]
    bG2p = padrows(bG2, FP)                      # [640]

    def asb(a):  # -> bf16 device array
        return np.ascontiguousarray(np.asarray(a, dtype=np.float32).astype(np_bf16))

    asf = lambda a: np.ascontiguousarray(a, dtype=np.float32)

    # SBUF layouts: partition dim first; K-chunks as middle axis.
    d["w1"] = asb(W1)                                            # [128, 256]
    d["w2"] = asb(W2p.reshape(2, 128, FP).transpose(1, 0, 2))    # [128, 2, 640]
    d["w2k1"] = asb(W2K1p.reshape(2, 128, FP).transpose(1, 0, 2))
    d["m2"] = asb(M2p.reshape(KC, 128, FP).transpose(1, 0, 2))   # [128, 5, 640]
    # C is padded to 128 output columns: a 34-wide lhsT makes the PE use a
    # 128x34 tile config, and the config switch back to 128x128 for the
    # next stage was measured to cost ~538ns every iteration.
    Cp128 = np.zeros((FP, 128), f64)
    Cp128[:, :OUTW] = Cp
    d["cw"] = asb(Cp128.reshape(KC, 128, 128).transpose(1, 0, 2))  # [128, 5, 128]
    d["b1"] = asf(b1.astype(f64).reshape(2, 128).T)              # [128, 2]
    d["bk1"] = asf(bK1p.reshape(KC, 128).T)                      # [128, 5]
    d["bg2"] = asf(bG2p.reshape(KC, 128).T)                      # [128, 5]
    d["bc"] = asf(bC.reshape(OUTW, 1))                           # [34, 1]
    return d


def _build_nc():
    """Build the per-core Bass program (same NEFF on all 8 cores)."""
    nc = bacc.Bacc("TRN2", target_bir_lowering=False, debug=False)

    xT = nc.dram_tensor("xT", [D, R_CORE], bf16, kind="ExternalInput").ap()
    w1 = nc.dram_tensor("w1", [128, HID], bf16, kind="ExternalInput").ap()
    w2 = nc.dram_tensor("w2", [128, 2, FP], bf16, kind="ExternalInput").ap()
    w2k1 = nc.dram_tensor("w2k1", [128, 2, FP], bf16, kind="ExternalInput").ap()
    m2 = nc.dram_tensor("m2", [128, KC, FP], bf16, kind="ExternalInput").ap()
    cw = nc.dram_tensor("cw", [128, KC, 128], bf16, kind="ExternalInput").ap()
    b1 = nc.dram_tensor("b1", [128, 2], f32, kind="ExternalInput").ap()
    bk1 = nc.dram_tensor("bk1", [128, KC], f32, kind="ExternalInput").ap()
    bg2 = nc.dram_tensor("bg2", [128, KC], f32, kind="ExternalInput").ap()
    bc = nc.dram_tensor("bc", [OUTW, 1], f32, kind="ExternalInput").ap()
    outT = nc.dram_tensor("outT", [OUTW, R_CORE], f32, kind="ExternalOutput").ap()

    with tile.TileContext(nc) as tc:
        with (
            tc.tile_pool(name="consts", bufs=1) as consts,
            tc.tile_pool(name="acts", bufs=2) as acts,
            tc.tile_pool(name="xio", bufs=3) as xio,
            tc.tile_pool(name="ps", bufs=1, space=bass.MemorySpace.PSUM) as ps,
        ):
            # All weights stream on GPSIMD's DMA queue (gpsimd is otherwise
            # idle): putting them on scalar's queue was measured to delay
            # tile 0's gelus ~4us (each DMA issue costs ~700ns of
            # engine-sequencer time), and sync's queue must stay free so
            # xt(0) issues immediately.
            # gpsimd queue: the weights tile 0 needs first (L1/L2).
            # Small constants ride gpsimd's SWDGE queue (~1us/descriptor but
            # tiny transfers); the three big slabs go on scalar's HWDGE
            # queue, whose 3 issue slots (~2.4us) clear before tile 0's
            # first gelu, so the scalar engine is not delayed and m2 lands
            # ~6us earlier than on the SWDGE queue.
            w1s = consts.tile([128, HID], bf16)
            nc.gpsimd.dma_start(w1s, w1)
            b1s = consts.tile([128, 2], f32)
            nc.gpsimd.dma_start(b1s, b1)
            bk1s = consts.tile([128, KC], f32)
            nc.gpsimd.dma_start(bk1s, bk1)
            bg2s = consts.tile([128, KC], f32)
            nc.gpsimd.dma_start(bg2s, bg2)
            cws = consts.tile([128, KC, 128], bf16)
            nc.gpsimd.dma_start(cws, cw)
            bcs = consts.tile([OUTW, 1], f32)
            nc.gpsimd.dma_start(bcs, bc)

            # Dummy 1-element gelu at the head of the scalar queue: forces
            # the 1.28us ACT_TABLE_LOAD during the DMA dead-time instead of
            # on tile 0's critical path (measured at 13.2us otherwise).
            scr = consts.tile([1, 2], f32)
            nc.vector.memset(scr, 0)
            nc.scalar.activation(scr[0:1, 1:2], scr[0:1, 0:1], GELU)

            # Big slabs are issued on the sync queue right after xt(0)
            # (see the t==0 branch in the loop): sync carries no
            # dependency-waiting instructions, so they stream immediately
            # and in parallel with gpsimd's small constants, while the
            # scalar queue stays free to run tile 0's gelus on time.
            w2k1s = consts.tile([128, 2, FP], bf16)
            w2s = consts.tile([128, 2, FP], bf16)
            m2s_w = consts.tile([128, KC, FP], bf16)

            def emit_gat2(p):
                """GAT2 for tile p: z2 = m1@M2 (psum), t2 = gelu(z2 + bG2),
                m2 = t2 + m1.  Runs one iteration after its m1 was made, so
                m1 is long ready and the PE streams without stalls."""
                m1s, p_t = p
                t2s = acts.tile([128, KC, TILE_N], bf16, tag="t2s")
                m2s = acts.tile([128, KC, TILE_N], bf16, tag="m2s", bufs=3)
                for m in range(KC):
                    pz = ps.tile([128, TILE_N], f32, tag="pp", bufs=7,
                                 name=f"pz_{p_t}_{m}")
                    for k in range(KC):
                        nc.tensor.matmul(pz, m2s_w[:, k, bass.ts(m, 128)],
                                         m1s[:, k, :],
                                         start=(k == 0), stop=(k == KC - 1))
                    nc.scalar.activation(t2s[:, m, :], pz, GELU,
                                         bias=bg2s[:, m : m + 1])
                    nc.vector.tensor_add(m2s[:, m, :], t2s[:, m, :], m1s[:, m, :])
                return m2s

            def emit_l5(p, store_on_sync=False):
                """out = m2 @ C + bC for tile p (two iterations late)."""
                m2s, p_sl, p_t = p
                po = ps.tile([128, TILE_N], f32, tag="po", bufs=1, name=f"po_{p_t}")
                for k in range(KC):
                    nc.tensor.matmul(po, cws[:, k, :], m2s[:, k, :],
                                     start=(k == 0), stop=(k == KC - 1))
                ot = xio.tile([OUTW, TILE_N], f32, tag="ot", name=f"ot_{p_t}")
                nc.vector.tensor_scalar_add(ot, po[0:OUTW, :], bcs)
                q = nc.sync if store_on_sync else nc.gpsimd
                q.dma_start(outT[:, p_sl], ot)

            prev1 = None   # tile awaiting GAT2
            prev2 = None   # tile awaiting L5
            for t in range(N_TILES):
                sl = bass.ts(t, TILE_N)

                xt = xio.tile([D, TILE_N], bf16, tag="xt", name=f"xt_{t}")
                nc.sync.dma_start(xt, xT[:, sl])
                if t == 0:
                    nc.sync.dma_start(w2k1s, w2k1)
                    nc.sync.dma_start(w2s, w2)
                    # m2 is split per k-chunk so GAT2(0)'s first groups can
                    # start as soon as their chunk lands instead of waiting
                    # for the whole 800KB slab (~1.5us of warm-up stalls).
                    for kk in range(KC):
                        nc.sync.dma_start(m2s_w[:, kk, :], m2[:, kk, :])

                # L1: hT = gelu(W1.T @ xT + b1)   [2 chunks of 128]
                # L1's PSUM joins the shared 7-deep rotation: a dedicated
                # 2-bank tile left the rotation at depth 5, where GAT2's
                # group-start matmuls measurably wait (~430ns/iteration)
                # for the vector add freeing their bank.
                hs = acts.tile([128, 2, TILE_N], bf16, tag="hs")
                for c in range(2):
                    ph = ps.tile([128, TILE_N], f32, tag="pp", bufs=7,
                                 name=f"ph_{t}_{c}")
                    nc.tensor.matmul(ph, w1s[:, bass.ts(c, 128)], xt,
                                     start=True, stop=True)
                    nc.scalar.activation(hs[:, c, :], ph, GELU,
                                         bias=b1s[:, c : c + 1])

                # GAT2(t-1) and L5(t-2) run RIGHT AFTER L1(t), before L2(t):
                # this way every pz group-start's PSUM bank tenant (a
                # pn0/pt1 from the PREVIOUS iteration) was freed ~an
                # iteration ago, instead of by an m1 add ~27ns earlier --
                # each just-in-time wait cost the PE a ~432ns pipeline
                # restart per iteration.
                if prev1 is not None:
                    m2s = emit_gat2(prev1)
                    if prev2 is not None:
                        emit_l5(prev2)
                    prev2 = (m2s, bass.ts(prev1[1], TILE_N), prev1[1])

                # L2b/L2a interleaved per output chunk:
                #   t1 = gelu(h @ W2K1 + bK1)   (GAT1 fused; t1 stays f32)
                #   m1 = t1 + h @ W2            (b2 deferred; bf16 out)
                t1s = acts.tile([128, KC, TILE_N], f32, tag="t1s")
                m1s = acts.tile([128, KC, TILE_N], bf16, tag="m1s", bufs=3)
                for m in range(KC):
                    pt1 = ps.tile([128, TILE_N], f32, tag="pp", bufs=7,
                                  name=f"pt1_{t}_{m}")
                    for k in range(2):
                        nc.tensor.matmul(pt1, w2k1s[:, k, bass.ts(m, 128)],
                                         hs[:, k, :], start=(k == 0), stop=(k == 1))
                    nc.scalar.activation(t1s[:, m, :], pt1, GELU,
                                         bias=bk1s[:, m : m + 1])
                    pn0 = ps.tile([128, TILE_N], f32, tag="pp", bufs=7,
                                  name=f"pn0_{t}_{m}")
                    for k in range(2):
                        nc.tensor.matmul(pn0, w2s[:, k, bass.ts(m, 128)],
                                         hs[:, k, :], start=(k == 0), stop=(k == 1))
                    nc.vector.tensor_add(m1s[:, m, :], t1s[:, m, :], pn0)

                prev1 = (m1s, t)

            # flush the pipeline tail (last two stores ride sync's HWDGE
            # queue -- idle by now, and its completion latency is lower
            # than gpsimd's SWDGE, shortening the drain)
            m2s = emit_gat2(prev1)
            if prev2 is not None:
                emit_l5(prev2, store_on_sync=True)
            emit_l5((m2s, bass.ts(N_TILES - 1, TILE_N), N_TILES - 1),
                    store_on_sync=True)

    nc.compile()
    return nc


_NC_CACHE = None


def _run(inputs: dict, trace: bool = False):
    global _NC_CACHE
    if _NC_CACHE is None:
        _NC_CACHE = _build_nc()
    nc = _NC_CACHE

    x = np.ascontiguousarray(inputs["x"], dtype=np.float32)
    consts = _prep_constants(
        *(np.asarray(inputs[k], dtype=np.float32)
          for k in ("W1", "b1", "W2", "b2", "adj1", "Wg1", "bg1",
                    "adj2", "Wg2", "bg2", "Wc", "bc"))
    )

    xflat = x.reshape(ROWS, D)
    in_maps = []
    for i in range(N_CORES):
        shard = np.ascontiguousarray(
            xflat[i * R_CORE : (i + 1) * R_CORE].T.astype(np_bf16)
        )
        m = {"xT": shard}
        m.update(consts)
        in_maps.append(m)

    res = run_bass_kernel_spmd(nc, in_maps, core_ids=list(range(N_CORES)), trace=trace)
    parts = [np.asarray(r["outT"]).T for r in res.results]     # each [8192, 34]
    out = np.concatenate(parts, axis=0).reshape(B, W, NN, 2)
    return np.ascontiguousarray(out, dtype=np.float32), res


def kernel(**inputs) -> np.ndarray:
    out, _ = _run(inputs, trace=False)
    return out
